# revision 17
# baseline (speedup 1.0000x reference)
"""AttnGatedCRFV2 loss on 8 Trainium2 NeuronCores.

Math (equivalent to the reference, no [B,HW,HW] intermediates):
    mask = (1 - eye(HW)) * kernel_xy_mask
    S[b] = Y_b @ Y_b^T             (Y_b = y_hat_softmax[b] as [HW, C])
    loss = (1/(HW*B)) * sum_b [ sum(mask*K_b) - 2*tr(Y_b^T (mask*K_b) Y_b)
                                + tr(Y_b^T mask Y_b) ]

Sharding: contraction rows i are split across 8 cores (288 rows each).
Each core reads kernel[:, i0:i0+288, :] (21.2MB) + mask[i0:i0+288, :]
(2.65MB) once, produces a scalar partial; the host sums 8 partials.

Device pipeline per core:
  - SWDGE cast-DMA kern tiles fp32->bf16
  - DVE bf16 multiply with resident mask rows (K' = kern * mask)
  - PE matmuls: stationary lhsT = [Y_b | 1 | 0] (M=32), moving = K' tiles,
    4 batches stacked into PSUM col-groups of shared [128, 512] banks
  - ACT copies PSUM->SBUF (cast bf16), freeing DVE and recycling PSUM fast
  - DVE product with host-folded weights ([-2*Y_b ; 1 ; 0] rows)
  - ones-vector reduce-matmuls accumulate everything into one [1,512] bank
    (C chain first, then the diagonal self-loop correction side chain on
    host-extracted diagonals via a negative-ones matmul, then the main
    halves last so the serial tail after the final kern tile is minimal)
  - final row reduce, scale by 1/(HW*B), DMA out

All kern DMAs are full-128-partition: the 8 per-batch leftover 32-row
strips load as contiguous [128, 576] reshapes (partition p = (i, j-quarter))
multiplied by an identically reshaped mask tile and contracted with a
quarter-routing block lhsT. Measured (loop-in-NEFF differential,
axon/PJRT): matches the DMA-only floor within ~0.6 us (58.8 vs 58.2 us
interleaved; 21.23 MB/core at ~330-440 GB/s/core effective, drifting with
ambient load). kb bufs=8, in-place mask-multiply, ACT-mediated PSUM
recycling, corr side-chain shares the C pool's bank.
"""
import numpy as np
import ml_dtypes
from contextlib import ExitStack

B, C, H, W = 8, 4, 48, 48
HW = H * W                    # 2304
NCORES = 8
SL = HW // NCORES             # 288 rows per core
ITS = (128, 128, 32)          # i-tile sizes (288 = 128+128+32)
JS = (512, 512, 512, 512, 256)  # j super-blocks (2304)
SCALE = 1.0 / float(HW * B)

_BUILT = None   # (nc, run) cache
LAST_RESULT = None


def _joff(k):
    return sum(JS[:k])


def _build(loop_n=None, kbufs=8):
    from concourse import bacc, tile, mybir

    f32, bf16 = mybir.dt.float32, mybir.dt.bfloat16
    AOT = mybir.AluOpType

    nc = bacc.Bacc("TRN2", target_bir_lowering=False, debug=False,
                   num_devices=NCORES)

    ks_ap = nc.dram_tensor("ks", [B, SL, HW], f32, kind="ExternalInput").ap()
    ms_ap = nc.dram_tensor("ms", [SL, HW], f32, kind="ExternalInput").ap()
    yx_ap = nc.dram_tensor("yx", [B, SL, 32], bf16, kind="ExternalInput").ap()
    yst_ap = nc.dram_tensor("yst", [SL, 32], bf16, kind="ExternalInput").ap()
    wa_ap = nc.dram_tensor("wa", [128, HW], bf16, kind="ExternalInput").ap()
    wb_ap = nc.dram_tensor("wb", [128, HW], bf16, kind="ExternalInput").ap()
    wc_ap = nc.dram_tensor("wc", [32, HW], bf16, kind="ExternalInput").ap()
    kd_ap = nc.dram_tensor("kd", [B, SL], f32, kind="ExternalInput").ap()
    md_ap = nc.dram_tensor("md", [B, SL], f32, kind="ExternalInput").ap()
    yd_ap = nc.dram_tensor("yd", [32, SL], f32, kind="ExternalInput").ap()
    sel_ap = nc.dram_tensor("sel", [32, B], f32, kind="ExternalInput").ap()
    no8_ap = nc.dram_tensor("no8", [B, 1], bf16, kind="ExternalInput").ap()
    o128_ap = nc.dram_tensor("o128", [128, 1], bf16, kind="ExternalInput").ap()
    o32_ap = nc.dram_tensor("o32", [32, 1], bf16, kind="ExternalInput").ap()
    yq_ap = nc.dram_tensor("yq", [B, 128, 32], bf16, kind="ExternalInput").ap()
    wql_ap = nc.dram_tensor("wql", [B, 32, 512], bf16, kind="ExternalInput").ap()
    wql2_ap = nc.dram_tensor("wql2", [B, 32, 64], bf16, kind="ExternalInput").ap()
    out_ap = nc.dram_tensor("partial", [1, 1], f32, kind="ExternalOutput").ap()

    with tile.TileContext(nc) as tc, ExitStack() as ctx:
        consts = ctx.enter_context(tc.tile_pool(name="consts", bufs=1))
        yxp = ctx.enter_context(tc.tile_pool(name="yxp", bufs=1))
        kbp = ctx.enter_context(tc.tile_pool(name="kbp", bufs=kbufs))
        work = ctx.enter_context(tc.tile_pool(name="work", bufs=4))
        cwork = ctx.enter_context(tc.tile_pool(name="cwork", bufs=2))
        small = ctx.enter_context(tc.tile_pool(name="small", bufs=1))
        r1ps = ctx.enter_context(tc.tile_pool(name="r1ps", bufs=5, space="PSUM"))
        lps = ctx.enter_context(tc.tile_pool(name="lps", bufs=1, space="PSUM"))
        cps = ctx.enter_context(tc.tile_pool(name="cps", bufs=1, space="PSUM"))
        sums = ctx.enter_context(tc.tile_pool(name="sums", bufs=1, space="PSUM"))

        # ---- resident constants ----
        maskt = []
        for it, isz in enumerate(ITS):
            r0 = 128 * it
            mt = consts.tile([isz, HW], bf16, tag=f"mask{it}")
            nc.gpsimd.dma_start(mt[:], ms_ap[r0 : r0 + isz, :])  # cast f32->bf16
            maskt.append(mt)
        wa = consts.tile([128, HW], bf16, tag="wa")
        nc.sync.dma_start(wa[:], wa_ap[:, :])
        wb = consts.tile([128, HW], bf16, tag="wb")
        nc.sync.dma_start(wb[:], wb_ap[:, :])
        wc = consts.tile([32, HW], bf16, tag="wc")
        nc.sync.dma_start(wc[:], wc_ap[:, :])
        yst = []
        for it, isz in enumerate(ITS):
            r0 = 128 * it
            t = consts.tile([isz, 32], bf16, tag=f"yst{it}")
            nc.sync.dma_start(t[:], yst_ap[r0 : r0 + isz, :])
            yst.append(t)
        yx = {}
        for b in range(B):
            for it, isz in enumerate(ITS):
                r0 = 128 * it
                t = yxp.tile([isz, 32], bf16, tag=f"yx{b}_{it}")
                nc.sync.dma_start(t[:], yx_ap[b, r0 : r0 + isz, :])
                yx[(b, it)] = t
        o128 = consts.tile([128, 1], bf16, tag="o128")
        nc.sync.dma_start(o128[:], o128_ap[:, :])
        o32 = consts.tile([32, 1], bf16, tag="o32")
        nc.sync.dma_start(o32[:], o32_ap[:, :])
        # last-32-rows mask, reshaped to a full-partition [128, 576] tile
        mq = consts.tile([128, 576], bf16, tag="mq")
        nc.gpsimd.dma_start(
            mq[:],
            ms_ap[256:288, :].rearrange("i j -> (i j)").rearrange(
                "(p f) -> p f", p=128))
        yq = {}
        wql = {}
        wql2 = {}
        for b in range(B):
            t = consts.tile([128, 32], bf16, tag=f"yq{b}")
            nc.sync.dma_start(t[:], yq_ap[b])
            yq[b] = t
            t2 = consts.tile([32, 512], bf16, tag=f"wql{b}")
            nc.sync.dma_start(t2[:], wql_ap[b])
            wql[b] = t2
            t3 = consts.tile([32, 64], bf16, tag=f"wql2{b}")
            nc.sync.dma_start(t3[:], wql2_ap[b])
            wql2[b] = t3
        no8 = consts.tile([B, 1], bf16, tag="no8")
        nc.sync.dma_start(no8[:], no8_ap[:, :])
        kd = consts.tile([B, SL], f32, tag="kd")
        nc.sync.dma_start(kd[:], kd_ap[:, :])
        md = consts.tile([B, SL], f32, tag="md")
        nc.sync.dma_start(md[:], md_ap[:, :])
        yd = consts.tile([32, SL], f32, tag="yd")
        nc.sync.dma_start(yd[:], yd_ap[:, :])
        sel = consts.tile([32, B], f32, tag="sel")
        nc.sync.dma_start(sel[:], sel_ap[:, :])

        def body():
            # single accumulating [1, 512] sum bank; every reduce-matmul lands here
            sumbank = sums.tile([1, 512], f32, name="sumbank")
            first_sum = [True]

            def sum_matmul(lhsT, rhs, nj, stop=False):
                nc.tensor.matmul(out=sumbank[0:1, 0:nj], lhsT=lhsT, rhs=rhs,
                                 start=first_sum[0], stop=stop)
                first_sum[0] = False

            # ---- C chain: term3 = tr(Ystack^T mask Ystack) over this i-slice ----
            for k, nj in enumerate(JS):
                j0 = _joff(k)
                pc = cps.tile([32, 512], f32, tag="cbank")
                for it, isz in enumerate(ITS):
                    nc.tensor.matmul(out=pc[:, 0:nj], lhsT=yst[it][:],
                                     rhs=maskt[it][:, j0 : j0 + nj],
                                     start=(it == 0), stop=(it == 2))
                csb = cwork.tile([32, 512], bf16, tag="csb")
                nc.scalar.copy(csb[:, 0:nj], pc[:, 0:nj])
                prc = cwork.tile([32, 512], bf16, tag="prc")
                nc.vector.tensor_tensor(out=prc[:, 0:nj], in0=csb[:, 0:nj],
                                        in1=wc[:, j0 : j0 + nj], op=AOT.mult)
                sum_matmul(o32[:], prc[:, 0:nj], nj)

            # ---- diagonal (self-loop) correction ----
            sq = small.tile([32, SL], f32, tag="sq")
            nc.vector.tensor_tensor(out=sq[:], in0=yd[:], in1=yd[:], op=AOT.mult)
            sd8 = cps.tile([B, SL], f32, tag="cbank", name="sd8")
            nc.tensor.matmul(out=sd8[:, :], lhsT=sel[:], rhs=sq[:],
                             start=True, stop=True)
            t1 = small.tile([B, SL], f32, tag="t1")
            nc.vector.tensor_tensor(out=t1[:], in0=kd[:], in1=sd8[:, :], op=AOT.mult)
            t1b = small.tile([B, SL], f32, tag="t1b")
            nc.vector.tensor_scalar_mul(t1b[:], t1[:], 2.0)
            t2 = small.tile([B, SL], f32, tag="t2")
            nc.vector.tensor_tensor(out=t2[:], in0=kd[:], in1=sd8[:, :], op=AOT.add)
            t3 = small.tile([B, SL], f32, tag="t3")
            nc.vector.tensor_tensor(out=t3[:], in0=t2[:], in1=t1b[:], op=AOT.subtract)
            ce2 = small.tile([B, SL], bf16, tag="ce2")
            nc.vector.tensor_tensor(out=ce2[:], in0=t3[:], in1=md[:], op=AOT.mult)
            sum_matmul(no8[:], ce2[:], SL)

            # ---- main: per batch, R1^T = [Y_b|1|0]^T (mask*K_b) ----
            for half in range(2):
                banks = [r1ps.tile([128, 512], f32, tag="bank", name=f"bank{half}_{k}")
                         for k in range(len(JS))]
                for g in range(4):
                    b = 4 * half + g
                    for it in (0, 1):
                        r0 = 128 * it
                        kp = kbp.tile([128, HW], bf16, tag="kb")
                        nc.gpsimd.dma_start(kp[:], ks_ap[b, r0 : r0 + 128, :])
                        nc.vector.tensor_tensor(out=kp[:], in0=kp[:],
                                                in1=maskt[it][:], op=AOT.mult)
                        for k, nj in enumerate(JS):
                            j0 = _joff(k)
                            nc.tensor.matmul(
                                out=banks[k][32 * g : 32 * g + 32, 0:nj],
                                lhsT=yx[(b, it)][:], rhs=kp[:, j0 : j0 + nj],
                                start=(it == 0), stop=(it == 1),
                                tile_position=(0, 32 * g),
                            )
                    # last 32 rows: contiguous reshape -> full-partition tile;
                    # partition p holds (i = 256+p//4, j-quarter q = p%4)
                    kq = kbp.tile([128, 576], bf16, tag="kq")
                    nc.gpsimd.dma_start(
                        kq[:],
                        ks_ap[b, 256:288, :].rearrange("i j -> (i j)").rearrange(
                            "(p f) -> p f", p=128))
                    nc.vector.tensor_tensor(out=kq[:], in0=kq[:], in1=mq[:],
                                            op=AOT.mult)
                    bankL = lps.tile([64, 512], f32, name=f"bankL{b}", tag="bankL")
                    nc.tensor.matmul(out=bankL[0:32, 0:512], lhsT=yq[b][:],
                                     rhs=kq[:, 0:512], start=True, stop=True,
                                     tile_position=(0, 0))
                    nc.tensor.matmul(out=bankL[32:64, 0:64], lhsT=yq[b][:],
                                     rhs=kq[:, 512:576], start=True, stop=True,
                                     tile_position=(0, 32))
                    lsb = work.tile([32, 512], bf16, tag="lsb")
                    nc.scalar.copy(lsb[:], bankL[0:32, 0:512])
                    prl = work.tile([32, 512], bf16, tag="prl")
                    nc.vector.tensor_tensor(out=prl[:], in0=lsb[:],
                                            in1=wql[b][:], op=AOT.mult)
                    sum_matmul(o32[:], prl[:], 512)
                    lsb2 = work.tile([32, 64], bf16, tag="lsb2")
                    nc.scalar.copy(lsb2[:], bankL[32:64, 0:64])
                    prl2 = work.tile([32, 64], bf16, tag="prl2")
                    nc.vector.tensor_tensor(out=prl2[:], in0=lsb2[:],
                                            in1=wql2[b][:], op=AOT.mult)
                    sum_matmul(o32[:], prl2[:], 64)
                wh = wa if half == 0 else wb
                for k, nj in enumerate(JS):
                    j0 = _joff(k)
                    sb = work.tile([128, 512], bf16, tag="sb")
                    nc.scalar.copy(sb[:, 0:nj], banks[k][:, 0:nj])
                    pr = work.tile([128, 512], bf16, tag="pr")
                    nc.vector.tensor_tensor(out=pr[:, 0:nj], in0=sb[:, 0:nj],
                                            in1=wh[:, j0 : j0 + nj], op=AOT.mult)
                    sum_matmul(o128[:], pr[:, 0:nj], nj,
                               stop=(half == 1 and k == len(JS) - 1))

            # ---- final reduce + scale + out ----
            fin = small.tile([1, 1], f32, tag="fin")
            nc.vector.tensor_reduce(out=fin[:, 0:1], in_=sumbank[0:1, :],
                                    axis=mybir.AxisListType.X, op=AOT.add)
            sc = small.tile([1, 1], f32, tag="sc")
            nc.scalar.mul(sc[:], fin[:], SCALE)
            nc.sync.dma_start(out_ap[:, :], sc[:])

        if loop_n is None:
            body()
        else:
            with tc.For_i(0, loop_n, 1):
                body()

    nc.compile()
    return nc


def _prep_inputs(y_hat_softmax, kern, mask):
    bf16 = ml_dtypes.bfloat16
    y = np.ascontiguousarray(np.asarray(y_hat_softmax, np.float32)).reshape(B, C, HW)
    kern = np.asarray(kern, np.float32)
    mask = np.asarray(mask, np.float32)

    yk = y.transpose(0, 2, 1)                       # [B, HW, C]
    yx_full = np.zeros((B, HW, 32), np.float32)
    yx_full[:, :, :C] = yk
    yx_full[:, :, 4] = 1.0
    yst_full = np.ascontiguousarray(yk.transpose(1, 0, 2)).reshape(HW, 32)

    wa = np.zeros((128, HW), np.float32)
    wb = np.zeros((128, HW), np.float32)
    for g in range(4):
        wa[32 * g : 32 * g + C] = -2.0 * y[g]
        wa[32 * g + 4] = 1.0
        wb[32 * g : 32 * g + C] = -2.0 * y[4 + g]
        wb[32 * g + 4] = 1.0
    wc = np.ascontiguousarray(y.reshape(32, HW))

    # leftover-rows (i in [256+i0r, 288+i0r) of the slice) quarter-routing:
    # lhsT yq[b][p, 8q+cx] = (p%4==q) * yext32[b, islice][256 + p//4, cx]
    # and weights wql[b] matching bankL layout: rows 8q+cx -> W_b[cx, 576q+f]
    # (f<512), rows 32+8q+cx -> W_b[cx, 576q+512+f'] (f'<64).
    sel = np.zeros((32, B), np.float32)
    for b in range(B):
        sel[4 * b : 4 * b + 4, b] = 1.0

    rep = {
        "wa": wa.astype(bf16), "wb": wb.astype(bf16), "wc": wc.astype(bf16),
        "sel": sel,
        "no8": np.full((B, 1), -1.0, bf16),
        "o128": np.ones((128, 1), bf16),
        "o32": np.ones((32, 1), bf16),
    }

    idx = np.arange(SL)
    in_maps = []
    for c in range(NCORES):
        i0 = SL * c
        sl = slice(i0, i0 + SL)
        m = dict(rep)
        m["ks"] = np.ascontiguousarray(kern[:, sl, :])
        m["ms"] = np.ascontiguousarray(mask[sl, :])
        m["yx"] = np.ascontiguousarray(yx_full[:, sl, :]).astype(bf16)
        m["yst"] = np.ascontiguousarray(yst_full[sl, :]).astype(bf16)
        m["kd"] = np.ascontiguousarray(kern[:, i0 + idx, i0 + idx])
        m["md"] = np.ascontiguousarray(
            np.broadcast_to(mask[i0 + idx, i0 + idx], (B, SL)))
        m["yd"] = np.ascontiguousarray(y[:, :, sl].reshape(32, SL))
        yx_sl = yx_full[:, sl, :]                     # [B, SL, 32] f32
        wfull = np.zeros((B, 5, HW), np.float32)      # folded weights per b
        wfull[:, :C, :] = -2.0 * y
        wfull[:, 4, :] = 1.0
        yq = np.zeros((B, 128, 32), np.float32)
        wql = np.zeros((B, 32, 512), np.float32)
        wql2 = np.zeros((B, 32, 64), np.float32)
        p = np.arange(128)
        for b in range(B):
            for cx in range(5):
                yq[b, p, 8 * (p % 4) + cx] = yx_sl[b, 256 + p // 4, cx]
            for q in range(4):
                for cx in range(5):
                    wql[b, 8 * q + cx, :] = wfull[b, cx, 576 * q : 576 * q + 512]
                    wql2[b, 8 * q + cx, :] = wfull[b, cx,
                                                   576 * q + 512 : 576 * (q + 1)]
        m["yq"] = yq.astype(bf16)
        m["wql"] = wql.astype(bf16)
        m["wql2"] = wql2.astype(bf16)
        in_maps.append(m)
    return in_maps


def kernel(y_hat_softmax, kernel, kernel_xy_mask, kernel_h, kernel_w):
    global _BUILT, LAST_RESULT
    from concourse.bass_utils import run_bass_kernel_spmd

    if _BUILT is None:
        _BUILT = _build()
    nc = _BUILT

    in_maps = _prep_inputs(y_hat_softmax, kernel, kernel_xy_mask)
    res = run_bass_kernel_spmd(nc, in_maps, list(range(NCORES)))
    LAST_RESULT = res
    total = np.float32(0.0)
    for i in range(NCORES):
        total += np.float32(res.results[i]["partial"][0, 0])
    return np.float32(total)

